# revision 26
# baseline (speedup 1.0000x reference)
"""Multi-head attention (B=8, N=1024, C=768, H=12, D=64) on 8 TRN2 NeuronCores.

Sharding: pure data parallel — one batch element per core, weights replicated,
no collectives. Each core computes its full attention block.

On-chip layout (per core), fp16 operands / fp32 PSUM accumulation:
  - host casts x / w_qkv / w_proj to fp16; x^T arrives via DMA xbar transpose
    (2-byte dtype) — no PE transposes at all.
  - qk^T [1536, N] = w_qk.T @ x^T (transposed activations; the q half is
    pre-scaled by 1/sqrt(D) during the PSUM->SBUF cast so exp needs no scale).
  - v [N, 768] natural = (x^T).T @ w_v, stored fp16 with a ones column per
    head ([128, 12, 65]) so attn@v also produces the softmax denominator.
  - heads processed in even/odd pairs: the pair's score matmuls use PE row
    groups 0-1 vs 2-3 (K=64 at base partitions 0/64) and can run
    concurrently; score output is a single-bank fp16 [128, 1024] PSUM tile
    (no accumulation), consumed by one wide ACT exp per m-tile.
  - attn@v: 4 accumulation chains (2 heads x 2 n-chunks) interleaved over
    m-tiles so consecutive matmuls target different PSUM banks.
  - softmax denominators: fp32 rowsum row -> base-0 copy ->
    reciprocal_approx_fast (~18 bits, plenty for well-conditioned sums) ->
    fp16 -> PE broadcast (ones[1,128].T @ recip[1,512]) -> DVE multiply.
  - final = (out^T).T @ w_proj + b: out^T stationary flips the result back to
    natural [N, C] so the output DMA is contiguous fp32.
"""

import numpy as np

B, N, C = 8, 1024, 768
H, D = 12, 64
F3 = 3 * C          # 2304
FQK = 2 * C         # 1536
SCALE = D ** -0.5   # 0.125
NT = N // 128       # 8 n-tiles / m-tiles
CT = C // 128       # 6 c-tiles
FT = FQK // 128     # 12 qk feature tiles
NCH = N // 512      # 2 psum chunks over n
VCH = 384           # v / proj free chunk (C = 2*384)

_compiled = None


def _build():
    import concourse.mybir as mybir
    import concourse.tile as tile
    from concourse import bacc
    from concourse.masks import make_identity

    f32 = mybir.dt.float32
    f16 = mybir.dt.float16

    nc = bacc.Bacc("TRN2", target_bir_lowering=False, debug=False)

    x_d = nc.dram_tensor("x", [N, C], f16, kind="ExternalInput").ap()
    wqkv_d = nc.dram_tensor("w_qkv", [C, F3], f16, kind="ExternalInput").ap()
    wproj_d = nc.dram_tensor("w_proj", [C, C], f16, kind="ExternalInput").ap()
    bias_d = nc.dram_tensor("b_bcast", [128, C], f32, kind="ExternalInput").ap()
    out_d = nc.dram_tensor("out", [N, C], f32, kind="ExternalOutput").ap()

    with tile.TileContext(nc) as tc:
        with tc.tile_pool(name="const", bufs=1) as const_pool:
            ones_f32 = const_pool.tile([1, 128], f32)
            nc.gpsimd.memset(ones_f32[:], 1.0)
            ones_sb = const_pool.tile([1, 128], f16)
            nc.vector.tensor_copy(ones_sb[:], ones_f32[:])
            vones_f32 = const_pool.tile([128, H], f32)
            nc.gpsimd.memset(vones_f32[:], 1.0)
            ident_f32 = const_pool.tile([128, 128], f32)
            make_identity(nc, ident_f32[:])
            ident = const_pool.tile([128, 128], f16)
            nc.vector.tensor_copy(ident[:], ident_f32[:])
            bias_sb = const_pool.tile([128, C], f32)
            nc.scalar.dma_start(bias_sb[:], bias_d)

            # ---- persistent activations ----
            with tc.tile_pool(name="acts", bufs=1) as acts:
                xT = [acts.tile([128, N], f16, tag=f"xT{ci}", name=f"xT{ci}")
                      for ci in range(CT)]
                qkT = [acts.tile([128, N], f16, tag=f"qkT{fi}", name=f"qkT{fi}")
                       for fi in range(FT)]
                vnat = [acts.tile([128, H, D + 1], f16, tag=f"v{ni}",
                                  name=f"v{ni}") for ni in range(NT)]
                onorm = [acts.tile([128, N], f16, tag=f"on{ci}", name=f"on{ci}")
                         for ci in range(CT)]

                with tc.tile_pool(name="wq", bufs=1) as wq_pool, \
                     tc.tile_pool(name="wp", bufs=1) as wp_pool, \
                     tc.tile_pool(name="xin", bufs=4) as xin_pool, \
                     tc.tile_pool(name="acc", bufs=2, space="PSUM") as acc_pool:
                    # ---- phase 0: load x, PE-transpose to x^T (fp16) ----
                    xt_ins = []
                    for ni in range(NT):
                        xt_in = xin_pool.tile([128, C], f16, tag="xt_in",
                                              name=f"xt_in{ni}")
                        xt_ins.append(xt_in)
                        nc.sync.dma_start(
                            xt_in[:], x_d[ni * 128:(ni + 1) * 128, :])
                    for ni in range(NT):
                        for ci in range(CT):
                            pt = acc_pool.tile([128, 128], f16, tag="acc",
                                               name=f"pt{ni}_{ci}")
                            nc.tensor.transpose(
                                pt[:], xt_ins[ni][:, ci * 128:(ci + 1) * 128],
                                ident[:])
                            nc.vector.tensor_copy(
                                xT[ci][:, ni * 128:(ni + 1) * 128], pt[:])
                    wq = [wq_pool.tile([128, F3], f16, tag=f"wq{ci}",
                                       name=f"wq{ci}") for ci in range(CT)]
                    for ci in range(CT):
                        eng = nc.scalar if ci < 5 else nc.sync
                        eng.dma_start(
                            wq[ci][:], wqkv_d[ci * 128:(ci + 1) * 128, :])
                    wp = [wp_pool.tile([128, C], f16, tag=f"wp{ci}",
                                       name=f"wp{ci}") for ci in range(CT)]
                    for ci in range(CT):
                        nc.scalar.dma_start(
                            wp[ci][:], wproj_d[ci * 128:(ci + 1) * 128, :])

                    def qk_proj(fi):
                        pqk = [acc_pool.tile([128, 512], f32, tag="acc",
                                             name=f"pqk{fi}_{ch}")
                               for ch in range(NCH)]
                        for ci in range(CT):
                            for ch in range(NCH):
                                nc.tensor.matmul(
                                    pqk[ch][:],
                                    wq[ci][:, fi * 128:(fi + 1) * 128],
                                    xT[ci][:, ch * 512:(ch + 1) * 512],
                                    start=(ci == 0), stop=(ci == CT - 1))
                        for ch in range(NCH):
                            if fi < 6:
                                # q half: fold in the 1/sqrt(D) scale
                                nc.vector.tensor_scalar_mul(
                                    qkT[fi][:, ch * 512:(ch + 1) * 512],
                                    pqk[ch][:], SCALE)
                            else:
                                nc.vector.tensor_copy(
                                    qkT[fi][:, ch * 512:(ch + 1) * 512],
                                    pqk[ch][:])

                    def v_proj(ni):
                        nc.vector.tensor_copy(vnat[ni][:, :, D], vones_f32[:])
                        pv = [acc_pool.tile([128, VCH], f32, tag="acc",
                                            name=f"pv{ni}_{vc}")
                              for vc in range(2)]
                        for ci in range(CT):
                            for vc in range(2):
                                nc.tensor.matmul(
                                    pv[vc][:],
                                    xT[ci][:, ni * 128:(ni + 1) * 128],
                                    wq[ci][:, FQK + vc * VCH:
                                           FQK + (vc + 1) * VCH],
                                    start=(ci == 0), stop=(ci == CT - 1))
                        for vc in range(2):
                            nc.vector.tensor_copy(
                                vnat[ni][:, vc * 6:(vc + 1) * 6, 0:D],
                                pv[vc][:].rearrange("p (h d) -> p h d", d=D))

                    # first pair's qk tiles
                    qk_proj(0)
                    qk_proj(6)

                    # ---- attention, head pairs, qk for pair j+1 interleaved
                    attn_pools = (
                        tc.tile_pool(name="fin", bufs=3),
                        tc.tile_pool(name="rc", bufs=2),
                        tc.tile_pool(name="exp", bufs=26),
                        tc.tile_pool(name="pss", bufs=2, space="PSUM"),
                        tc.tile_pool(name="pso", bufs=2, space="PSUM"),
                    )
                    fin_pool, rc_pool, exp_pool, pss_pool, pso_pool = [
                        p.__enter__() for p in attn_pools]

                    def scores_exp(j, exp_t):
                        pair = (2 * j, 2 * j + 1)
                        for mi in range(NT):
                            for h in pair:
                                qrow = (h % 2) * D
                                ps = pss_pool.tile([128, N], f32, tag="pss",
                                                   name=f"pss{h}_{mi}")
                                for ch in range(NCH):
                                    nc.tensor.matmul(
                                        ps[:, ch * 512:(ch + 1) * 512],
                                        qkT[6 + h // 2][qrow:qrow + D,
                                                        mi * 128:(mi + 1) * 128],
                                        qkT[h // 2][qrow:qrow + D,
                                                    ch * 512:(ch + 1) * 512],
                                        start=True, stop=True)
                                et = exp_pool.tile([128, N], f16, tag="exp",
                                                   name=f"exp{h}_{mi}")
                                nc.scalar.activation(
                                    et[:], ps[:],
                                    mybir.ActivationFunctionType.Exp)
                                exp_t[h].append(et)

                    def attnv_norm(j, exp_t):
                        pair = (2 * j, 2 * j + 1)
                        for ch in range(NCH):
                            po = {}
                            for h in pair:
                                po[h] = pso_pool.tile(
                                    [D + 1, 512], f32, tag="po",
                                    name=f"po{h}_{ch}")
                            for mi in range(NT):
                                for h in pair:
                                    nc.tensor.matmul(
                                        po[h][:],
                                        vnat[mi][:, h, :],
                                        exp_t[h][mi][:,
                                                     ch * 512:(ch + 1) * 512],
                                        start=(mi == 0), stop=(mi == NT - 1))
                            for h in pair:
                                orow = (h % 2) * D
                                p = po[h]
                                rs = rc_pool.tile([1, 512], f32, tag="rs",
                                                  name=f"rs{h}_{ch}", bufs=2)
                                nc.vector.tensor_copy(rs[:], p[D:D + 1, :])
                                rcf = rc_pool.tile([1, 512], f32, tag="rcf",
                                                   name=f"rcf{h}_{ch}", bufs=2)
                                nc.vector.reciprocal_approx_fast(rcf[:], rs[:])
                                rc = rc_pool.tile([1, 512], f16, tag="rc",
                                                  name=f"rc{h}_{ch}", bufs=2)
                                nc.vector.tensor_copy(rc[:], rcf[:])
                                ou = rc_pool.tile([D, 512], f16, tag="ou",
                                                  name=f"ou{h}_{ch}", bufs=2)
                                nc.vector.tensor_copy(ou[:], p[0:D, :])
                                pb = pso_pool.tile([128, 512], f32, tag="po",
                                                   name=f"pb{h}_{ch}")
                                nc.tensor.matmul(
                                    pb[:], ones_sb[:], rc[:],
                                    start=True, stop=True)
                                nc.vector.tensor_mul(
                                    onorm[h // 2][orow:orow + D,
                                                  ch * 512:(ch + 1) * 512],
                                    ou[:], pb[0:D, :])

                    def proj_part(ci_lo, ci_hi):
                        first = ci_lo == 0
                        for ni in range(NT):
                            pf = [acc_pool.tile([128, VCH], f32, tag="acc",
                                                name=f"pf{ni}_{fc}_{ci_lo}")
                                  for fc in range(2)]
                            for ci in range(ci_lo, ci_hi):
                                for fc in range(2):
                                    nc.tensor.matmul(
                                        pf[fc][:],
                                        onorm[ci][:, ni * 128:(ni + 1) * 128],
                                        wp[ci][:, fc * VCH:(fc + 1) * VCH],
                                        start=(ci == ci_lo),
                                        stop=(ci == ci_hi - 1))
                            fin = fin_pool.tile([128, C], f32, tag="fin",
                                                name=f"fin{ni}_{ci_lo}")
                            for fc in range(2):
                                sl = slice(fc * VCH, (fc + 1) * VCH)
                                if first:
                                    nc.vector.tensor_add(
                                        fin[:, sl], pf[fc][:], bias_sb[:, sl])
                                else:
                                    nc.vector.tensor_copy(fin[:, sl], pf[fc][:])
                            if first:
                                nc.sync.dma_start(
                                    out_d[ni * 128:(ni + 1) * 128, :], fin[:])
                            else:
                                nc.gpsimd.dma_start(
                                    out_d[ni * 128:(ni + 1) * 128, :], fin[:],
                                    accum_op=mybir.AluOpType.add)

                    exp_ts = {}
                    for j in range(H // 2):
                        exp_ts[j] = {2 * j: [], 2 * j + 1: []}
                        scores_exp(j, exp_ts[j])
                        if j == 0:
                            for ni in range(NT):
                                v_proj(ni)
                        if j + 1 < H // 2:
                            qk_proj(j + 1)
                            qk_proj(6 + j + 1)
                        if j >= 1:
                            attnv_norm(j - 1, exp_ts.pop(j - 1))
                        if j == 4:
                            proj_part(0, 4)
                    attnv_norm(H // 2 - 1, exp_ts.pop(H // 2 - 1))
                    proj_part(4, CT)

                    for p in reversed(attn_pools):
                        p.__exit__(None, None, None)

    nc.compile()
    return nc


def _get_compiled():
    global _compiled
    if _compiled is None:
        _compiled = _build()
    return _compiled


def _run(x, w_qkv, w_proj, b_proj, **kwargs):
    from concourse.bass_utils import run_bass_kernel_spmd

    x = np.asarray(x, dtype=np.float32).astype(np.float16)
    w_qkv = np.ascontiguousarray(
        np.asarray(w_qkv, dtype=np.float32).astype(np.float16))
    w_proj = np.ascontiguousarray(
        np.asarray(w_proj, dtype=np.float32).astype(np.float16))
    b_bcast = np.ascontiguousarray(
        np.broadcast_to(np.asarray(b_proj, dtype=np.float32), (128, C)))

    nc = _get_compiled()
    in_maps = [
        {"x": np.ascontiguousarray(x[b]), "w_qkv": w_qkv,
         "w_proj": w_proj, "b_bcast": b_bcast}
        for b in range(B)
    ]
    return run_bass_kernel_spmd(nc, in_maps, core_ids=list(range(B)), **kwargs)


def kernel(x, w_qkv, w_proj, b_proj, **_):
    res = _run(x, w_qkv, w_proj, b_proj)
    return np.stack([res.results[b]["out"] for b in range(B)], axis=0)


# revision 27
# speedup vs baseline: 1.0103x; 1.0103x over previous
"""Multi-head attention (B=8, N=1024, C=768, H=12, D=64) on 8 TRN2 NeuronCores.

Sharding: pure data parallel — one batch element per core, weights replicated,
no collectives. Each core computes its full attention block.

On-chip layout (per core), fp16 operands / fp32 PSUM accumulation:
  - host casts x / w_qkv / w_proj to fp16; x^T via PE transpose-mode matmuls
    (fp16, 1 cyc/row; DMA xbar-transpose was slower — it serializes against
    every other DMA due to the xbar-mode hazard).
  - qk^T [1536, N] = w_qk.T @ x^T (transposed activations; the q half is
    pre-scaled by 1/sqrt(D) during the PSUM->SBUF cast so exp needs no scale).
  - v [N, 768] natural = (x^T).T @ w_v, stored fp16 with a ones column per
    head ([128, 12, 65]) so attn@v also produces the softmax denominator in
    row 64 of the same matmul (costs nothing: matmul time is N cycles,
    independent of M).
  - heads processed in even/odd pairs with a one-pair software-pipeline lag:
    scores/exp of pair j+1 overlap attn@v of pair j so ACT (the exp engine,
    ~107us busy) never starves; qk^T projection of pair j+1 is interleaved as
    PE filler, which also keeps the PE HAM clock-gate warm. The pair's score
    matmuls (K=64, base partitions 0/64) land in different PE row groups and
    run concurrently. Scores accumulate into a 2-bank fp32 [128, 1024] PSUM
    tile consumed by one wide ACT exp per m-tile (halves ACT op count).
  - attn@v: accumulation chains (head x chunk) interleaved so consecutive
    matmuls target different PSUM banks (hides the drain).
  - softmax denominators: fp32 rowsum row -> base-0 copy ->
    reciprocal_approx_fast (~18 bits, plenty for well-conditioned sums; the
    exact DVE reciprocal costs 3.3us per row) -> fp16 -> PE broadcast
    (ones[1,128].T @ recip[1,512]) -> DVE multiply.
  - final = (out^T).T @ w_proj + b: out^T stationary flips the result back to
    natural [N, C] so the output DMA is contiguous fp32. The projection is
    split: head-pairs 0-3 are projected and DMA'd while pairs 4-5 still run;
    the ci 4-5 remainder lands via an accumulating gpsimd DMA (avoids an
    aliased in-place DVE add, which corrupted one first-run).
"""

import numpy as np

B, N, C = 8, 1024, 768
H, D = 12, 64
F3 = 3 * C          # 2304
FQK = 2 * C         # 1536
SCALE = D ** -0.5   # 0.125
NT = N // 128       # 8 n-tiles / m-tiles
CT = C // 128       # 6 c-tiles
FT = FQK // 128     # 12 qk feature tiles
NCH = N // 512      # 2 psum chunks over n
VCH = 384           # v / proj free chunk (C = 2*384)

_compiled = None


def _build():
    import concourse.mybir as mybir
    import concourse.tile as tile
    from concourse import bacc
    from concourse.masks import make_identity

    f32 = mybir.dt.float32
    f16 = mybir.dt.float16

    nc = bacc.Bacc("TRN2", target_bir_lowering=False, debug=False)

    x_d = nc.dram_tensor("x", [N, C], f16, kind="ExternalInput").ap()
    wqkv_d = nc.dram_tensor("w_qkv", [C, F3], f16, kind="ExternalInput").ap()
    wproj_d = nc.dram_tensor("w_proj", [C, C], f16, kind="ExternalInput").ap()
    bias_d = nc.dram_tensor("b_bcast", [128, C], f32, kind="ExternalInput").ap()
    out_d = nc.dram_tensor("out", [N, C], f32, kind="ExternalOutput").ap()

    with tile.TileContext(nc) as tc:
        with tc.tile_pool(name="const", bufs=1) as const_pool:
            ones_f32 = const_pool.tile([1, 128], f32)
            nc.gpsimd.memset(ones_f32[:], 1.0)
            ones_sb = const_pool.tile([1, 128], f16)
            nc.vector.tensor_copy(ones_sb[:], ones_f32[:])
            vones_f32 = const_pool.tile([128, H], f32)
            nc.gpsimd.memset(vones_f32[:], 1.0)
            ident_f32 = const_pool.tile([128, 128], f32)
            make_identity(nc, ident_f32[:])
            ident = const_pool.tile([128, 128], f16)
            nc.vector.tensor_copy(ident[:], ident_f32[:])
            bias_sb = const_pool.tile([128, C], f32)
            nc.scalar.dma_start(bias_sb[:], bias_d)

            # ---- persistent activations ----
            with tc.tile_pool(name="acts", bufs=1) as acts:
                xT = [acts.tile([128, N], f16, tag=f"xT{ci}", name=f"xT{ci}")
                      for ci in range(CT)]
                qkT = [acts.tile([128, N], f16, tag=f"qkT{fi}", name=f"qkT{fi}")
                       for fi in range(FT)]
                vnat = [acts.tile([128, H, D + 1], f16, tag=f"v{ni}",
                                  name=f"v{ni}") for ni in range(NT)]
                onorm = [acts.tile([128, N], f16, tag=f"on{ci}", name=f"on{ci}")
                         for ci in range(CT)]

                with tc.tile_pool(name="wq", bufs=1) as wq_pool, \
                     tc.tile_pool(name="wp", bufs=1) as wp_pool, \
                     tc.tile_pool(name="xin", bufs=4) as xin_pool, \
                     tc.tile_pool(name="acc", bufs=2, space="PSUM") as acc_pool:
                    # ---- phase 0: load x, PE-transpose to x^T (fp16) ----
                    xt_ins = []
                    for ni in range(NT):
                        xt_in = xin_pool.tile([128, C], f16, tag="xt_in",
                                              name=f"xt_in{ni}")
                        xt_ins.append(xt_in)
                        nc.sync.dma_start(
                            xt_in[:], x_d[ni * 128:(ni + 1) * 128, :])
                    for ni in range(NT):
                        for ci in range(CT):
                            pt = acc_pool.tile([128, 128], f16, tag="acc",
                                               name=f"pt{ni}_{ci}")
                            nc.tensor.transpose(
                                pt[:], xt_ins[ni][:, ci * 128:(ci + 1) * 128],
                                ident[:])
                            nc.vector.tensor_copy(
                                xT[ci][:, ni * 128:(ni + 1) * 128], pt[:])
                    wq = [wq_pool.tile([128, F3], f16, tag=f"wq{ci}",
                                       name=f"wq{ci}") for ci in range(CT)]
                    for ci in range(CT):
                        eng = nc.scalar if ci < 5 else nc.sync
                        eng.dma_start(
                            wq[ci][:], wqkv_d[ci * 128:(ci + 1) * 128, :])
                    wp = [wp_pool.tile([128, C], f16, tag=f"wp{ci}",
                                       name=f"wp{ci}") for ci in range(CT)]
                    for ci in range(CT):
                        nc.scalar.dma_start(
                            wp[ci][:], wproj_d[ci * 128:(ci + 1) * 128, :])

                    def qk_proj(fi):
                        pqk = [acc_pool.tile([128, 512], f32, tag="acc",
                                             name=f"pqk{fi}_{ch}")
                               for ch in range(NCH)]
                        for ci in range(CT):
                            for ch in range(NCH):
                                nc.tensor.matmul(
                                    pqk[ch][:],
                                    wq[ci][:, fi * 128:(fi + 1) * 128],
                                    xT[ci][:, ch * 512:(ch + 1) * 512],
                                    start=(ci == 0), stop=(ci == CT - 1))
                        for ch in range(NCH):
                            if fi < 6:
                                # q half: fold in the 1/sqrt(D) scale
                                nc.vector.tensor_scalar_mul(
                                    qkT[fi][:, ch * 512:(ch + 1) * 512],
                                    pqk[ch][:], SCALE)
                            else:
                                nc.vector.tensor_copy(
                                    qkT[fi][:, ch * 512:(ch + 1) * 512],
                                    pqk[ch][:])

                    def v_proj(ni):
                        nc.vector.tensor_copy(vnat[ni][:, :, D], vones_f32[:])
                        pv = [acc_pool.tile([128, VCH], f32, tag="acc",
                                            name=f"pv{ni}_{vc}")
                              for vc in range(2)]
                        for ci in range(CT):
                            for vc in range(2):
                                nc.tensor.matmul(
                                    pv[vc][:],
                                    xT[ci][:, ni * 128:(ni + 1) * 128],
                                    wq[ci][:, FQK + vc * VCH:
                                           FQK + (vc + 1) * VCH],
                                    start=(ci == 0), stop=(ci == CT - 1))
                        for vc in range(2):
                            nc.vector.tensor_copy(
                                vnat[ni][:, vc * 6:(vc + 1) * 6, 0:D],
                                pv[vc][:].rearrange("p (h d) -> p h d", d=D))

                    # first pair's qk tiles
                    qk_proj(0)
                    qk_proj(6)

                    # ---- attention, head pairs, qk for pair j+1 interleaved
                    attn_pools = (
                        tc.tile_pool(name="fin", bufs=3),
                        tc.tile_pool(name="rc", bufs=2),
                        tc.tile_pool(name="exp", bufs=26),
                        tc.tile_pool(name="pss", bufs=2, space="PSUM"),
                        tc.tile_pool(name="pso", bufs=2, space="PSUM"),
                    )
                    fin_pool, rc_pool, exp_pool, pss_pool, pso_pool = [
                        p.__enter__() for p in attn_pools]

                    def scores_exp(j, exp_t):
                        pair = (2 * j, 2 * j + 1)
                        for mi in range(NT):
                            for h in pair:
                                qrow = (h % 2) * D
                                ps = pss_pool.tile([128, N], f32, tag="pss",
                                                   name=f"pss{h}_{mi}")
                                for ch in range(NCH):
                                    nc.tensor.matmul(
                                        ps[:, ch * 512:(ch + 1) * 512],
                                        qkT[6 + h // 2][qrow:qrow + D,
                                                        mi * 128:(mi + 1) * 128],
                                        qkT[h // 2][qrow:qrow + D,
                                                    ch * 512:(ch + 1) * 512],
                                        start=True, stop=True)
                                et = exp_pool.tile([128, N], f16, tag="exp",
                                                   name=f"exp{h}_{mi}")
                                nc.scalar.activation(
                                    et[:], ps[:],
                                    mybir.ActivationFunctionType.Exp)
                                exp_t[h].append(et)

                    def attnv_norm(j, exp_t):
                        pair = (2 * j, 2 * j + 1)
                        for ch in range(NCH):
                            po = {}
                            for h in pair:
                                po[h] = pso_pool.tile(
                                    [D + 1, 512], f32, tag="po",
                                    name=f"po{h}_{ch}")
                            for mi in range(NT):
                                for h in pair:
                                    nc.tensor.matmul(
                                        po[h][:],
                                        vnat[mi][:, h, :],
                                        exp_t[h][mi][:,
                                                     ch * 512:(ch + 1) * 512],
                                        start=(mi == 0), stop=(mi == NT - 1))
                            for h in pair:
                                orow = (h % 2) * D
                                p = po[h]
                                rs = rc_pool.tile([1, 512], f32, tag="rs",
                                                  name=f"rs{h}_{ch}", bufs=2)
                                nc.vector.tensor_copy(rs[:], p[D:D + 1, :])
                                rcf = rc_pool.tile([1, 512], f32, tag="rcf",
                                                   name=f"rcf{h}_{ch}", bufs=2)
                                nc.vector.reciprocal_approx_fast(rcf[:], rs[:])
                                rc = rc_pool.tile([1, 512], f16, tag="rc",
                                                  name=f"rc{h}_{ch}", bufs=2)
                                nc.vector.tensor_copy(rc[:], rcf[:])
                                ou = rc_pool.tile([D, 512], f16, tag="ou",
                                                  name=f"ou{h}_{ch}", bufs=2)
                                nc.vector.tensor_copy(ou[:], p[0:D, :])
                                pb = pso_pool.tile([128, 512], f32, tag="po",
                                                   name=f"pb{h}_{ch}")
                                nc.tensor.matmul(
                                    pb[:], ones_sb[:], rc[:],
                                    start=True, stop=True)
                                nc.vector.tensor_mul(
                                    onorm[h // 2][orow:orow + D,
                                                  ch * 512:(ch + 1) * 512],
                                    ou[:], pb[0:D, :])

                    def proj_part(ci_lo, ci_hi):
                        first = ci_lo == 0
                        for ni in range(NT):
                            pf = [acc_pool.tile([128, VCH], f32, tag="acc",
                                                name=f"pf{ni}_{fc}_{ci_lo}")
                                  for fc in range(2)]
                            for ci in range(ci_lo, ci_hi):
                                for fc in range(2):
                                    nc.tensor.matmul(
                                        pf[fc][:],
                                        onorm[ci][:, ni * 128:(ni + 1) * 128],
                                        wp[ci][:, fc * VCH:(fc + 1) * VCH],
                                        start=(ci == ci_lo),
                                        stop=(ci == ci_hi - 1))
                            fin = fin_pool.tile([128, C], f32, tag="fin",
                                                name=f"fin{ni}_{ci_lo}")
                            for fc in range(2):
                                sl = slice(fc * VCH, (fc + 1) * VCH)
                                if first:
                                    nc.vector.tensor_add(
                                        fin[:, sl], pf[fc][:], bias_sb[:, sl])
                                else:
                                    nc.vector.tensor_copy(fin[:, sl], pf[fc][:])
                            if first:
                                nc.sync.dma_start(
                                    out_d[ni * 128:(ni + 1) * 128, :], fin[:])
                            else:
                                nc.gpsimd.dma_start(
                                    out_d[ni * 128:(ni + 1) * 128, :], fin[:],
                                    accum_op=mybir.AluOpType.add)

                    exp_ts = {}
                    for j in range(H // 2):
                        exp_ts[j] = {2 * j: [], 2 * j + 1: []}
                        scores_exp(j, exp_ts[j])
                        if j == 0:
                            for ni in range(NT):
                                v_proj(ni)
                        if j + 1 < H // 2:
                            qk_proj(j + 1)
                            qk_proj(6 + j + 1)
                        if j >= 1:
                            attnv_norm(j - 1, exp_ts.pop(j - 1))
                        if j == 4:
                            proj_part(0, 4)
                    attnv_norm(H // 2 - 1, exp_ts.pop(H // 2 - 1))
                    proj_part(4, CT)

                    for p in reversed(attn_pools):
                        p.__exit__(None, None, None)

    nc.compile()
    return nc


def _get_compiled():
    global _compiled
    if _compiled is None:
        _compiled = _build()
    return _compiled


def _run(x, w_qkv, w_proj, b_proj, **kwargs):
    from concourse.bass_utils import run_bass_kernel_spmd

    x = np.asarray(x, dtype=np.float32).astype(np.float16)
    w_qkv = np.ascontiguousarray(
        np.asarray(w_qkv, dtype=np.float32).astype(np.float16))
    w_proj = np.ascontiguousarray(
        np.asarray(w_proj, dtype=np.float32).astype(np.float16))
    b_bcast = np.ascontiguousarray(
        np.broadcast_to(np.asarray(b_proj, dtype=np.float32), (128, C)))

    nc = _get_compiled()
    in_maps = [
        {"x": np.ascontiguousarray(x[b]), "w_qkv": w_qkv,
         "w_proj": w_proj, "b_bcast": b_bcast}
        for b in range(B)
    ]
    return run_bass_kernel_spmd(nc, in_maps, core_ids=list(range(B)), **kwargs)


def kernel(x, w_qkv, w_proj, b_proj, **_):
    res = _run(x, w_qkv, w_proj, b_proj)
    return np.stack([res.results[b]["out"] for b in range(B)], axis=0)
